# revision 1
# baseline (speedup 1.0000x reference)
import sys

sys.path.insert(0, "/opt/trn_rl_repo")

import numpy as np

import concourse.bass as bass
import concourse.bacc as bacc
import concourse.mybir as mybir
import concourse.tile as tile
from concourse.bass_utils import run_bass_kernel_spmd

F32 = mybir.dt.float32
BF16 = mybir.dt.bfloat16

# Problem constants
B, C, Dn, Hn, Wn = 2, 64, 64, 64, 64
H2, HID = 256, 128
KT, DIL, PAD = 3, 3, 3
EPS = 1e-5

# Sharding: 8 cores = 2 samples x 4 D-chunks of 16 slices; halo 3 each side.
NCORES = 8
JD = 4          # D-chunks per sample
DCH = Dn // JD  # 16 own d slices per core
DHL = DCH + 2 * PAD  # 22 local (haloed) d slices
# H tiling: 2 blocks of 32 own rows, processed with 3-row halo (35 input rows).
HB, HBS = 2, Hn // 2
HIN = HBS + PAD          # 35 input rows per block
H2ROWS = HBS + 2 * PAD   # 38 rows stored in h2 ring tile
WPD = Wn + 2 * PAD       # 70 padded W
NSP = Dn * Hn * Wn       # spatial size per sample
NTOT = C * NSP           # GN reduction size per sample

TAPS = [(tz, ty, tx) for tz in (-1, 0, 1) for ty in (-1, 0, 1) for tx in (-1, 0, 1)]
HALO_DLS = [0, 1, 2, DHL - 3, DHL - 2, DHL - 1]

_CACHED = {}


def _build_nc(use_collectives=True):
    nc = bacc.Bacc(None, num_devices=NCORES)

    x_ext = nc.declare_dram_parameter("x", [C, DHL, Hn, Wn], F32, isOutput=False)
    at_ext = nc.declare_dram_parameter("at", [C, H2], F32, isOutput=False)       # (pre_w*gamma).T
    qrow_ext = nc.declare_dram_parameter("qrow", [1, H2], F32, isOutput=False)   # pre_b + pre_w@beta
    trow_ext = nc.declare_dram_parameter("trow", [1, H2], F32, isOutput=False)   # row sums of A
    diag_ext = nc.declare_dram_parameter("dwdiag", [128, 56 * 128], BF16, isOutput=False)
    scaw_ext = nc.declare_dram_parameter("sca_wT", [HID, HID], BF16, isOutput=False)
    scab_ext = nc.declare_dram_parameter("sca_b", [HID, 1], F32, isOutput=False)
    postw_ext = nc.declare_dram_parameter("post_wT", [HID, C], BF16, isOutput=False)
    pb_ext = nc.declare_dram_parameter("pb", [C, 1], F32, isOutput=False)
    mask_ext = nc.declare_dram_parameter("mask", [1, DHL], F32, isOutput=False)
    out_ext = nc.declare_dram_parameter("out", [C, DCH, Hn, Wn], F32, isOutput=True)

    groups = [[0, 1, 2, 3], [4, 5, 6, 7]]
    mm = mybir.AluOpType.mult
    aa = mybir.AluOpType.add

    with tile.TileContext(nc) as tc:
        with (
            tc.tile_pool(name="wts", bufs=1) as wp,
            tc.tile_pool(name="small", bufs=1) as sp,
            tc.tile_pool(name="dram", bufs=1, space="DRAM") as dp,
        ):
            # ---- persistent weight tiles ----
            at_t = wp.tile([C, H2], F32, tag="at")
            qrow_t = wp.tile([1, H2], F32, tag="qrow")
            trow_t = wp.tile([1, H2], F32, tag="trow")
            diag_t = wp.tile([128, 56 * 128], BF16, tag="diag")
            scaw_t = wp.tile([HID, HID], BF16, tag="scaw")
            scab_t = wp.tile([HID, 1], F32, tag="scab")
            postw_t = wp.tile([HID, C], BF16, tag="postw")
            pb_t = wp.tile([C, 1], F32, tag="pb")
            mask_t = wp.tile([1, DHL], F32, tag="mask")
            ones_t = wp.tile([128, 512], BF16, tag="ones")
            pool_cols = wp.tile([HID, 128], F32, tag="poolc")

            nc.sync.dma_start(at_t[:], at_ext[:])
            nc.sync.dma_start(qrow_t[:], qrow_ext[:])
            nc.sync.dma_start(trow_t[:], trow_ext[:])
            nc.sync.dma_start(diag_t[:], diag_ext[:])
            nc.sync.dma_start(scaw_t[:], scaw_ext[:])
            nc.sync.dma_start(scab_t[:], scab_ext[:])
            nc.sync.dma_start(postw_t[:], postw_ext[:])
            nc.sync.dma_start(pb_t[:], pb_ext[:])
            nc.sync.dma_start(mask_t[:], mask_ext[:])
            nc.gpsimd.memset(ones_t[:], 1.0)

            # ---- stage 1: GroupNorm stats over own region ----
            sum_cols = sp.tile([128, 8], F32, tag="sumc")
            sq_cols = sp.tile([128, 8], F32, tag="sqc")
            with tc.tile_pool(name="stats", bufs=3) as stp:
                for i in range(8):
                    xt = stp.tile([128, Hn * Wn], F32, tag="sx")
                    # rows = (c, d-pair) interleave; order irrelevant for sums
                    for k in range(2):
                        nc.sync.dma_start(
                            xt[64 * k : 64 * k + 64, :],
                            x_ext[:, PAD + 2 * i + k, :, :].rearrange(
                                "c h w -> c (h w)"
                            ),
                        )
                    scr = stp.tile([128, Hn * Wn], F32, tag="scr")
                    flat = xt[:]
                    nc.vector.tensor_reduce(
                        sum_cols[:, i : i + 1], flat, mybir.AxisListType.X, aa
                    )
                    nc.vector.tensor_mul(scr[:], flat, flat)
                    nc.vector.tensor_reduce(
                        sq_cols[:, i : i + 1], scr[:], mybir.AxisListType.X, aa
                    )

            both = sp.tile([128, 2], F32, tag="both")
            nc.vector.tensor_reduce(both[:, 0:1], sum_cols[:], mybir.AxisListType.X, aa)
            nc.vector.tensor_reduce(both[:, 1:2], sq_cols[:], mybir.AxisListType.X, aa)
            # partition-sum via ones-matmul: [1,2] = ones[128,1].T @ both[128,2]
            ones_c = sp.tile([128, 1], F32, tag="onesc")
            nc.gpsimd.memset(ones_c[:], 1.0)
            part = sp.tile([1, 2], F32, tag="part")
            with tc.tile_pool(name="stpsum", bufs=1, space="PSUM") as stps:
                pps_ = stps.tile([1, 2], F32, tag="stp")
                nc.tensor.matmul(pps_[:], ones_c[:], both[:])
                nc.vector.tensor_copy(part[:], pps_[:])

            st_in = dp.tile([1, 2], F32, tag="stin")
            st_out = dp.tile([1, 2], F32, tag="stout")
            nc.sync.dma_start(st_in[:], part[:])
            if use_collectives:
                nc.gpsimd.collective_compute(
                    "AllReduce", aa, replica_groups=groups,
                    ins=[st_in.opt()], outs=[st_out.opt()],
                )
            else:
                nc.sync.dma_start(st_out[:], st_in[:])
            tot = sp.tile([1, 2], F32, tag="tot")
            nc.sync.dma_start(tot[:], st_out[:])

            # ---- derive mu, r = rsqrt(var+eps); fold into pre-conv weights ----
            mu = sp.tile([1, 1], F32, tag="mu")
            e2 = sp.tile([1, 1], F32, tag="e2")
            nc.vector.tensor_scalar_mul(mu[:], tot[:, 0:1], 1.0 / NTOT)
            nc.vector.tensor_scalar_mul(e2[:], tot[:, 1:2], 1.0 / NTOT)
            mu2 = sp.tile([1, 1], F32, tag="mu2")
            nc.vector.tensor_mul(mu2[:], mu[:], mu[:])
            v = sp.tile([1, 1], F32, tag="v")
            nc.vector.tensor_sub(v[:], e2[:], mu2[:])
            nc.vector.tensor_scalar_add(v[:], v[:], EPS)
            sq = sp.tile([1, 1], F32, tag="sqv")
            nc.scalar.sqrt(sq[:], v[:])
            r0 = sp.tile([1, 1], F32, tag="r0")
            nc.vector.reciprocal(r0[:], sq[:])
            # one Newton step: r = r0 * (1.5 - 0.5 * v * r0^2)
            z = sp.tile([1, 1], F32, tag="z")
            nc.vector.tensor_mul(z[:], r0[:], r0[:])
            nc.vector.tensor_mul(z[:], z[:], v[:])
            nc.vector.tensor_scalar(z[:], z[:], -0.5, 1.5, mm, aa)
            r_ = sp.tile([1, 1], F32, tag="r_")
            nc.vector.tensor_mul(r_[:], r0[:], z[:])
            nrmu = sp.tile([1, 1], F32, tag="nrmu")
            nc.vector.tensor_mul(nrmu[:], r_[:], mu[:])
            nc.vector.tensor_scalar_mul(nrmu[:], nrmu[:], -1.0)

            ones_row = sp.tile([1, 128], F32, tag="onesr")
            nc.gpsimd.memset(ones_row[:], 1.0)
            r_b = sp.tile([C, 1], F32, tag="r_b")
            with tc.tile_pool(name="bcpsum", bufs=2, space="PSUM") as bcp:
                rbp = bcp.tile([C, 1], F32, tag="rbp")
                nc.tensor.matmul(rbp[:], ones_row[:, 0:C], r_[:])
                nc.vector.tensor_copy(r_b[:], rbp[:])

            # pre-conv lhsT [65, 256]: rows 0-63 = r*A^T, row 64 = q - r*mu*t
            lhsT_main = sp.tile([C + 1, H2], BF16, tag="lhsTm")
            nc.vector.tensor_scalar(lhsT_main[0:C, :], at_t[:], r_b[:], None, mm)
            nc.vector.scalar_tensor_tensor(
                lhsT_main[C : C + 1, :], trow_t[:], nrmu[:], qrow_t[:], mm, aa
            )
            lhsT_by_dl = {}
            with tc.tile_pool(name="mbpsum", bufs=2, space="PSUM") as mbp:
                for dl in range(DHL):
                    if dl in HALO_DLS:
                        mbps = mbp.tile([C + 1, 1], F32, tag="mbps")
                        nc.tensor.matmul(
                            mbps[:], ones_row[:, 0 : C + 1], mask_t[:, dl : dl + 1]
                        )
                        mb = sp.tile([C + 1, 1], F32, tag=f"mb{dl}")
                        nc.vector.tensor_copy(mb[:], mbps[:])
                        lv = sp.tile([C + 1, H2], BF16, tag=f"lv{dl}")
                        nc.vector.tensor_scalar(lv[:], lhsT_main[:], mb[:], None, mm)
                        lhsT_by_dl[dl] = lv
                    else:
                        lhsT_by_dl[dl] = lhsT_main

            h3_dram = dp.tile([HID, DCH, Hn * Wn], BF16, tag="h3d")

            # ---- stage 2: pre-conv -> dw conv -> gate -> pool, H-tiled pipeline ----
            with (
                tc.tile_pool(name="xin", bufs=3) as xp,
                tc.tile_pool(name="ring", bufs=8) as rp,
                tc.tile_pool(name="h3", bufs=4) as h3p,
                tc.tile_pool(name="prepsum", bufs=3, space="PSUM") as pps,
                tc.tile_pool(name="dwpsum", bufs=4, space="PSUM") as dps,
            ):
                for hb in range(HB):
                    h0 = hb * HBS - PAD  # first input h row (may be <0)
                    rs = PAD if hb == 0 else 0  # valid-row start in the 38-row tile
                    xh0 = max(h0, 0)
                    ring = {}
                    for dl in range(DHL):
                        # load x [64, 35, 64] + ones row
                        xt = xp.tile([C, HIN, Wn], F32, tag="xt")
                        nc.sync.dma_start(
                            xt[:, :, :], x_ext[:, dl, xh0 : xh0 + HIN, :]
                        )
                        xb = xp.tile([C + 1, HIN, Wn], BF16, tag="xb")
                        if dl % 2 == 0:
                            nc.vector.tensor_copy(xb[0:C, :, :], xt[:])
                        else:
                            nc.scalar.copy(xb[0:C, :, :], xt[:])
                        nc.gpsimd.memset(xb[C : C + 1, :, :], 1.0)

                        # h2 ring tile [128, 38, 70] bf16 per group
                        t_g = []
                        for g in range(2):
                            t3 = rp.tile([128, H2ROWS, WPD], BF16, tag=f"ring{g}")
                            # zero W pads and invalid h rows
                            nc.gpsimd.memset(t3[:, :, 0:PAD], 0.0)
                            nc.gpsimd.memset(t3[:, :, PAD + Wn :], 0.0)
                            if hb == 0:
                                nc.gpsimd.memset(t3[:, 0:PAD, PAD : PAD + Wn], 0.0)
                            else:
                                nc.gpsimd.memset(t3[:, H2ROWS - PAD :, PAD : PAD + Wn], 0.0)
                            t_g.append(t3)

                        lhsT = lhsT_by_dl[dl]
                        rows_per_chunk = [8, 8, 8, 8, 3]
                        rc0 = 0
                        for ci, nr in enumerate(rows_per_chunk):
                            rhs = xb[:, rc0 : rc0 + nr, :]
                            for g in range(2):
                                ps = pps.tile([128, 512], F32, tag="pps")
                                nc.tensor.matmul(
                                    ps[:, : nr * Wn], lhsT[:, g * 128 : (g + 1) * 128], rhs
                                )
                                dest = t_g[g][:, rs + rc0 : rs + rc0 + nr, PAD : PAD + Wn]
                                if (ci + g) % 2 == 0:
                                    nc.scalar.copy(dest, ps[:, : nr * Wn])
                                else:
                                    nc.vector.tensor_copy(dest, ps[:, : nr * Wn])
                            rc0 += nr

                        ring[dl] = t_g

                        if dl >= 6:
                            dl0 = dl - 6  # own-d index 0..15
                            h3t = h3p.tile([HID, HBS * Wn], BF16, tag="h3t")
                            for ci in range(4):  # chunks of 8 output h rows
                                oh = ci * 8
                                gp = []
                                for g in range(2):
                                    ps = dps.tile([128, 512], F32, tag="dps")
                                    for ti, (tz, ty, tx) in enumerate(TAPS):
                                        src = ring[dl + (tz - 1) * 3][g]
                                        r0_ = oh + PAD + ty * 3
                                        rhs = src[
                                            :,
                                            r0_ : r0_ + 8,
                                            PAD + tx * 3 : PAD + tx * 3 + Wn,
                                        ]
                                        di = (ti * 2 + g) * 128
                                        nc.tensor.matmul(
                                            ps[:],
                                            diag_t[:, di : di + 128],
                                            rhs,
                                            start=(ti == 0),
                                            stop=False,
                                        )
                                    db = (54 + g) * 128
                                    nc.tensor.matmul(
                                        ps[:],
                                        diag_t[:, db : db + 128],
                                        ones_t[:],
                                        start=False,
                                        stop=True,
                                    )
                                    gp.append(ps)
                                col = (hb * DCH + dl0) * 4 + ci
                                a_sb = h3p.tile([HID, 512], F32, tag="asb")
                                if ci % 2 == 0:
                                    nc.scalar.copy(a_sb[:], gp[0][:])
                                else:
                                    nc.vector.tensor_copy(a_sb[:], gp[0][:])
                                nc.vector.scalar_tensor_tensor(
                                    h3t[:, oh * Wn : (oh + 8) * Wn],
                                    gp[1][:], 1.0, a_sb[:], mm, mm,
                                    accum_out=pool_cols[:, col : col + 1],
                                )
                            nc.sync.dma_start(
                                h3_dram[:, dl0, hb * HBS * Wn : (hb + 1) * HBS * Wn],
                                h3t[:],
                            )

            # ---- stage 3: SCA pool allreduce -> attn -> fold into post weights ----
            pool_p = sp.tile([HID, 1], F32, tag="poolp")
            nc.vector.tensor_reduce(pool_p[:], pool_cols[:], mybir.AxisListType.X, aa)
            pl_in = dp.tile([HID, 1], F32, tag="plin")
            pl_out = dp.tile([HID, 1], F32, tag="plout")
            nc.sync.dma_start(pl_in[:], pool_p[:])
            if use_collectives:
                nc.gpsimd.collective_compute(
                    "AllReduce", aa, replica_groups=groups,
                    ins=[pl_in.opt()], outs=[pl_out.opt()],
                )
            else:
                nc.sync.dma_start(pl_out[:], pl_in[:])
            pool_f = sp.tile([HID, 1], F32, tag="poolf")
            nc.sync.dma_start(pool_f[:], pl_out[:])
            pool_bf = sp.tile([HID, 1], BF16, tag="poolbf")
            nc.vector.tensor_copy(pool_bf[:], pool_f[:])

            attn = sp.tile([HID, 1], F32, tag="attn")
            with tc.tile_pool(name="scapsum", bufs=1, space="PSUM") as scp:
                aps = scp.tile([HID, 1], F32, tag="aps")
                nc.tensor.matmul(aps[:], scaw_t[:], pool_bf[:])
                nc.scalar.activation(
                    attn[:], aps[:], mybir.ActivationFunctionType.Identity,
                    bias=scab_t[:], scale=1.0,
                )
            post_lhsT = sp.tile([HID, C], BF16, tag="postl")
            nc.vector.tensor_scalar(post_lhsT[:], postw_t[:], attn[:], None, mm)

            # ---- stage 4: post-conv + bias + residual ----
            with (
                tc.tile_pool(name="h3in", bufs=4) as hip,
                tc.tile_pool(name="xres", bufs=4) as xrp,
                tc.tile_pool(name="outp", bufs=4) as op_,
                tc.tile_pool(name="postpsum", bufs=2, space="PSUM") as ppo,
            ):
                for dl0 in range(DCH):
                    h3i = hip.tile([HID, Hn * Wn], BF16, tag="h3i")
                    nc.sync.dma_start(h3i[:], h3_dram[:, dl0, :])
                    xr = xrp.tile([C, Hn, Wn], F32, tag="xr")
                    nc.sync.dma_start(xr[:], x_ext[:, PAD + dl0, :, :])
                    ot = op_.tile([C, Hn * Wn], F32, tag="ot")
                    xrf = xr[:].rearrange("p a b -> p (a b)")
                    for half in range(2):
                        ps = ppo.tile([C, 2048], F32, tag="ppo")
                        for q4 in range(4):
                            c0 = half * 2048 + q4 * 512
                            nc.tensor.matmul(
                                ps[:, q4 * 512 : (q4 + 1) * 512],
                                post_lhsT[:],
                                h3i[:, c0 : c0 + 512],
                            )
                        nc.vector.scalar_tensor_tensor(
                            ot[:, half * 2048 : (half + 1) * 2048],
                            ps[:], pb_t[:], xrf[:, half * 2048 : (half + 1) * 2048],
                            aa, aa,
                        )
                    nc.sync.dma_start(
                        out_ext[:, dl0, :, :],
                        ot[:].rearrange("p (a b) -> p a b", b=Wn),
                    )

    nc.finalize()
    return nc


def _host_prep(inputs):
    x = np.asarray(inputs["x"], np.float32)
    gam = np.asarray(inputs["gn_gamma"], np.float32)
    bet = np.asarray(inputs["gn_beta"], np.float32)
    pre_w = np.asarray(inputs["pre_w"], np.float32)
    pre_b = np.asarray(inputs["pre_b"], np.float32)
    ddc_w = np.asarray(inputs["ddc_w"], np.float32).reshape(H2, KT * KT * KT)
    ddc_b = np.asarray(inputs["ddc_b"], np.float32)
    sca_w = np.asarray(inputs["sca_w"], np.float32)
    sca_b = np.asarray(inputs["sca_b"], np.float32)
    post_w = np.asarray(inputs["post_w"], np.float32)
    post_b = np.asarray(inputs["post_b"], np.float32)

    A = pre_w * gam[None, :]                    # [256, 64]
    at = np.ascontiguousarray(A.T)              # [64, 256]
    qrow = (pre_b + pre_w @ bet)[None, :]       # [1, 256]
    trow = A.sum(axis=1)[None, :]               # [1, 256]

    # diag matrices: [128, 56*128] bf16; tap ti group g at col block ti*2+g
    diag = np.zeros((128, 56, 128), np.float32)
    for ti in range(27):
        for g in range(2):
            idx = np.arange(128)
            diag[idx, ti * 2 + g, idx] = ddc_w[g * 128 : (g + 1) * 128, ti]
    for g in range(2):
        idx = np.arange(128)
        diag[idx, 54 + g, idx] = ddc_b[g * 128 : (g + 1) * 128]
    diag = diag.reshape(128, 56 * 128)

    import ml_dtypes

    bf = lambda a: a.astype(ml_dtypes.bfloat16)
    common = {
        "at": at,
        "qrow": qrow,
        "trow": trow,
        "dwdiag": bf(diag),
        "sca_wT": bf(np.ascontiguousarray((sca_w / NSP).T)),
        "sca_b": sca_b[:, None].astype(np.float32),
        "post_wT": bf(np.ascontiguousarray(post_w.T)),
        "pb": post_b[:, None].astype(np.float32),
    }

    in_maps = []
    for core in range(NCORES):
        b, j = core // JD, core % JD
        lo, hi = j * DCH - PAD, j * DCH + DCH + PAD
        xs = np.zeros((C, DHL, Hn, Wn), np.float32)
        clo, chi = max(lo, 0), min(hi, Dn)
        xs[:, clo - lo : chi - lo] = x[b, :, clo:chi]
        mask = np.ones((1, DHL), np.float32)
        for dl in range(DHL):
            dg = lo + dl
            if dg < 0 or dg >= Dn:
                mask[0, dl] = 0.0
        m = dict(common)
        m["x"] = xs
        m["mask"] = mask
        in_maps.append(m)
    return in_maps


def kernel(**inputs):
    if "nc" not in _CACHED:
        _CACHED["nc"] = _build_nc()
    nc = _CACHED["nc"]
    in_maps = _host_prep(inputs)
    res = run_bass_kernel_spmd(nc, in_maps, list(range(NCORES)))
    out = np.zeros((B, C, Dn, Hn, Wn), np.float32)
    for core in range(NCORES):
        b, j = core // JD, core % JD
        out[b, :, j * DCH : (j + 1) * DCH] = np.asarray(res.results[core]["out"])
    return out



# revision 11
# speedup vs baseline: 1.8475x; 1.8475x over previous
import sys

sys.path.insert(0, "/opt/trn_rl_repo")

import numpy as np

import concourse.bass as bass
import concourse.bacc as bacc
import concourse.mybir as mybir
import concourse.tile as tile
from concourse.bass_utils import run_bass_kernel_spmd

F32 = mybir.dt.float32
BF16 = mybir.dt.bfloat16
FP8 = mybir.dt.float8e4
DR = mybir.MatmulPerfMode.DoubleRow

# Problem constants
B, C, Dn, Hn, Wn = 2, 64, 64, 64, 64
H2, HID = 256, 128
KT, DIL, PAD = 3, 3, 3
EPS = 1e-5

# Sharding: 8 cores = 2 samples x 4 D-chunks of 16 slices; halo 3 each side.
NCORES = 8
JD = 4          # D-chunks per sample
DCH = Dn // JD  # 16 own d slices per core
DHL = DCH + 2 * PAD  # 22 local (haloed) d slices
# H tiling: 2 blocks of 32 own rows, processed with 3-row halo (35 input rows).
HB, HBS = 2, Hn // 2
HIN = HBS + PAD          # 35 input rows per block
H2ROWS = HBS + 2 * PAD   # 38 rows stored in h2 ring tile
WPD = Wn + 2 * PAD       # 70 padded W
NSP = Dn * Hn * Wn       # spatial size per sample
NTOT = C * NSP           # GN reduction size per sample

TAPS = [(tz, ty, tx) for tz in (-1, 0, 1) for ty in (-1, 0, 1) for tx in (-1, 0, 1)]
HALO_DLS = [0, 1, 2, DHL - 3, DHL - 2, DHL - 1]

_CACHED = {}


def _build_nc(use_collectives=True):
    nc = bacc.Bacc(None, num_devices=NCORES)

    x_ext = nc.declare_dram_parameter("x", [C, DHL, Hn, Wn], F32, isOutput=False)
    at_ext = nc.declare_dram_parameter("at", [C, H2], F32, isOutput=False)       # (pre_w*gamma).T
    qrow_ext = nc.declare_dram_parameter("qrow", [1, H2], F32, isOutput=False)   # pre_b + pre_w@beta
    trow_ext = nc.declare_dram_parameter("trow", [1, H2], F32, isOutput=False)   # row sums of A
    diag_ext = nc.declare_dram_parameter("dwdiag", [128, 56, 128], FP8, isOutput=False)
    scaw_ext = nc.declare_dram_parameter("sca_wT", [HID, HID], BF16, isOutput=False)
    scab_ext = nc.declare_dram_parameter("sca_b", [HID, 1], F32, isOutput=False)
    postw_ext = nc.declare_dram_parameter("post_wT", [HID, C], BF16, isOutput=False)
    pb_ext = nc.declare_dram_parameter("pb", [C, 1], F32, isOutput=False)
    mask_ext = nc.declare_dram_parameter("mask", [1, DHL], F32, isOutput=False)
    out_ext = nc.declare_dram_parameter("out", [C, DCH, Hn, Wn], F32, isOutput=True)

    groups = [[0, 1, 2, 3], [4, 5, 6, 7]]
    mm = mybir.AluOpType.mult
    aa = mybir.AluOpType.add

    with tile.TileContext(nc) as tc:
        with (
            tc.tile_pool(name="wts", bufs=1) as wp,
            tc.tile_pool(name="small", bufs=1) as sp,
            tc.tile_pool(name="dram", bufs=1, space="DRAM") as dp,
        ):
            # ---- persistent weight tiles ----
            at_t = wp.tile([C, H2], F32, tag="at")
            qrow_t = wp.tile([1, H2], F32, tag="qrow")
            trow_t = wp.tile([1, H2], F32, tag="trow")
            diag_t = wp.tile([128, 56, 128], FP8, tag="diag")
            scaw_t = wp.tile([HID, HID], BF16, tag="scaw")
            scab_t = wp.tile([HID, 1], F32, tag="scab")
            postw_t = wp.tile([HID, C], BF16, tag="postw")
            pb_t = wp.tile([C, 1], F32, tag="pb")
            mask_t = wp.tile([1, DHL], F32, tag="mask")
            ones_t = wp.tile([128, 512], FP8, tag="ones")
            pool_cols = wp.tile([HID, 128], F32, tag="poolc")

            nc.sync.dma_start(at_t[:], at_ext[:])
            nc.sync.dma_start(qrow_t[:], qrow_ext[:])
            nc.sync.dma_start(trow_t[:], trow_ext[:])
            nc.sync.dma_start(diag_t[:], diag_ext[:])
            nc.sync.dma_start(scaw_t[:], scaw_ext[:])
            nc.sync.dma_start(scab_t[:], scab_ext[:])
            nc.sync.dma_start(postw_t[:], postw_ext[:])
            nc.sync.dma_start(pb_t[:], pb_ext[:])
            nc.sync.dma_start(mask_t[:], mask_ext[:])
            nc.gpsimd.memset(ones_t[:], 1.0)

            # ---- stage 1: GroupNorm stats over own region ----
            sum_cols = sp.tile([128, 8], F32, tag="sumc")
            sq_cols = sp.tile([128, 8], F32, tag="sqc")
            with tc.tile_pool(name="stats", bufs=3) as stp:
                for i in range(8):
                    xt = stp.tile([128, Hn * Wn], F32, tag="sx")
                    # rows = (c, d-pair) interleave; order irrelevant for sums
                    for k in range(2):
                        nc.sync.dma_start(
                            xt[64 * k : 64 * k + 64, :],
                            x_ext[:, PAD + 2 * i + k, :, :].rearrange(
                                "c h w -> c (h w)"
                            ),
                        )
                    scr = stp.tile([128, Hn * Wn], F32, tag="scr")
                    flat = xt[:]
                    nc.vector.tensor_reduce(
                        sum_cols[:, i : i + 1], flat, mybir.AxisListType.X, aa
                    )
                    nc.vector.tensor_mul(scr[:], flat, flat)
                    nc.vector.tensor_reduce(
                        sq_cols[:, i : i + 1], scr[:], mybir.AxisListType.X, aa
                    )

            both = sp.tile([128, 2], F32, tag="both")
            nc.vector.tensor_reduce(both[:, 0:1], sum_cols[:], mybir.AxisListType.X, aa)
            nc.vector.tensor_reduce(both[:, 1:2], sq_cols[:], mybir.AxisListType.X, aa)
            # partition-sum via ones-matmul: [1,2] = ones[128,1].T @ both[128,2]
            ones_c = sp.tile([128, 1], F32, tag="onesc")
            nc.gpsimd.memset(ones_c[:], 1.0)
            part = sp.tile([1, 2], F32, tag="part")
            with tc.tile_pool(name="stpsum", bufs=1, space="PSUM") as stps:
                pps_ = stps.tile([1, 2], F32, tag="stp")
                nc.tensor.matmul(pps_[:], ones_c[:], both[:])
                nc.vector.tensor_copy(part[:], pps_[:])

            st_in = dp.tile([1, 2], F32, tag="stin")
            st_out = dp.tile([1, 2], F32, tag="stout")
            nc.sync.dma_start(st_in[:], part[:])
            if use_collectives:
                nc.gpsimd.collective_compute(
                    "AllReduce", aa, replica_groups=groups,
                    ins=[st_in.opt()], outs=[st_out.opt()],
                )
            else:
                nc.sync.dma_start(st_out[:], st_in[:])
            tot = sp.tile([1, 2], F32, tag="tot")
            nc.sync.dma_start(tot[:], st_out[:])

            # ---- derive mu, r = rsqrt(var+eps); fold into pre-conv weights ----
            mu = sp.tile([1, 1], F32, tag="mu")
            e2 = sp.tile([1, 1], F32, tag="e2")
            nc.vector.tensor_scalar_mul(mu[:], tot[:, 0:1], 1.0 / NTOT)
            nc.vector.tensor_scalar_mul(e2[:], tot[:, 1:2], 1.0 / NTOT)
            mu2 = sp.tile([1, 1], F32, tag="mu2")
            nc.vector.tensor_mul(mu2[:], mu[:], mu[:])
            v = sp.tile([1, 1], F32, tag="v")
            nc.vector.tensor_sub(v[:], e2[:], mu2[:])
            nc.vector.tensor_scalar_add(v[:], v[:], EPS)
            sq = sp.tile([1, 1], F32, tag="sqv")
            nc.scalar.sqrt(sq[:], v[:])
            r0 = sp.tile([1, 1], F32, tag="r0")
            nc.vector.reciprocal(r0[:], sq[:])
            # one Newton step: r = r0 * (1.5 - 0.5 * v * r0^2)
            z = sp.tile([1, 1], F32, tag="z")
            nc.vector.tensor_mul(z[:], r0[:], r0[:])
            nc.vector.tensor_mul(z[:], z[:], v[:])
            nc.vector.tensor_scalar(z[:], z[:], -0.5, 1.5, mm, aa)
            r_ = sp.tile([1, 1], F32, tag="r_")
            nc.vector.tensor_mul(r_[:], r0[:], z[:])
            nrmu = sp.tile([1, 1], F32, tag="nrmu")
            nc.vector.tensor_mul(nrmu[:], r_[:], mu[:])
            nc.vector.tensor_scalar_mul(nrmu[:], nrmu[:], -1.0)

            ones_row = sp.tile([1, 128], F32, tag="onesr")
            nc.gpsimd.memset(ones_row[:], 1.0)
            r_b = sp.tile([C, 1], F32, tag="r_b")
            with tc.tile_pool(name="bcpsum", bufs=2, space="PSUM") as bcp:
                rbp = bcp.tile([C, 1], F32, tag="rbp")
                nc.tensor.matmul(rbp[:], ones_row[:, 0:C], r_[:])
                nc.vector.tensor_copy(r_b[:], rbp[:])

            # pre-conv lhsT [65, 256]: rows 0-63 = r*A^T, row 64 = q - r*mu*t
            lhsT_main = sp.tile([C + 1, H2], BF16, tag="lhsTm")
            nc.vector.tensor_scalar(lhsT_main[0:C, :], at_t[:], r_b[:], None, mm)
            nc.vector.scalar_tensor_tensor(
                lhsT_main[C : C + 1, :], trow_t[:], nrmu[:], qrow_t[:], mm, aa
            )
            lhsT_by_dl = {}
            with tc.tile_pool(name="mbpsum", bufs=2, space="PSUM") as mbp:
                for dl in range(DHL):
                    if dl in HALO_DLS:
                        mbps = mbp.tile([C + 1, 1], F32, tag="mbps")
                        nc.tensor.matmul(
                            mbps[:], ones_row[:, 0 : C + 1], mask_t[:, dl : dl + 1]
                        )
                        mb = sp.tile([C + 1, 1], F32, tag=f"mb{dl}")
                        nc.vector.tensor_copy(mb[:], mbps[:])
                        lv = sp.tile([C + 1, H2], BF16, tag=f"lv{dl}")
                        nc.vector.tensor_scalar(lv[:], lhsT_main[:], mb[:], None, mm)
                        lhsT_by_dl[dl] = lv
                    else:
                        lhsT_by_dl[dl] = lhsT_main

            h3_dram = dp.tile([HID, DCH, Hn * Wn], BF16, tag="h3d")

            # ---- stage 2: pre-conv -> dw conv -> gate -> pool, H-tiled pipeline ----
            with (
                tc.tile_pool(name="xin", bufs=2) as xp,
                tc.tile_pool(name="ring", bufs=7) as rp,
                tc.tile_pool(name="h3", bufs=4) as h3p,
                tc.tile_pool(name="prepsum", bufs=3, space="PSUM") as pps,
                tc.tile_pool(name="dwpsum", bufs=4, space="PSUM") as dps,
            ):
                for hb in range(HB):
                    h0 = hb * HBS - PAD  # first input h row (may be <0)
                    rs = PAD if hb == 0 else 0  # valid-row start in the 38-row tile
                    xh0 = max(h0, 0)
                    ring = {}
                    for dl in range(DHL):
                        # load x [64, 35, 64] + ones row
                        xt = xp.tile([C, HIN, Wn], F32, tag="xt")
                        nc.sync.dma_start(
                            xt[:, :, :], x_ext[:, dl, xh0 : xh0 + HIN, :]
                        )
                        xb = xp.tile([C + 1, HIN, Wn], BF16, tag="xb")
                        if dl % 2 == 0:
                            nc.vector.tensor_copy(xb[0:C, :, :], xt[:])
                        else:
                            nc.scalar.copy(xb[0:C, :, :], xt[:])
                        nc.gpsimd.memset(xb[C : C + 1, :, :], 1.0)

                        # h2 ring tile [128, 3, 38, 70] fp8 per group.
                        # slot1 = h2 (standard); slot0 = h2 shifted down 6 rows
                        # (A[r] = s0[r-6]); slot2 = h2 shifted left 6 cols
                        # (s2[r,c] = s0[r,c+6]).
                        t_g = []
                        for g in range(2):
                            t3 = rp.tile([128, 3, H2ROWS, WPD], FP8, tag=f"ring{g}")
                            # W pads for slots 0,1; right-edge zeros for slot2
                            nc.gpsimd.memset(t3[:, 0:2, :, 0:PAD], 0.0)
                            nc.gpsimd.memset(t3[:, 0:2, :, PAD + Wn :], 0.0)
                            nc.gpsimd.memset(t3[:, 2, :, PAD + Wn - 6 :], 0.0)
                            if hb == 0:
                                nc.gpsimd.memset(t3[:, 1, 0:PAD, PAD : PAD + Wn], 0.0)
                                nc.gpsimd.memset(t3[:, 0, PAD + 3 : PAD + 6, PAD : PAD + Wn], 0.0)
                                nc.gpsimd.memset(t3[:, 2, 0:PAD, 0 : PAD + Wn - 6], 0.0)
                            else:
                                nc.gpsimd.memset(t3[:, 1, H2ROWS - PAD :, PAD : PAD + Wn], 0.0)
                                nc.gpsimd.memset(t3[:, 2, H2ROWS - PAD :, 0 : PAD + Wn - 6], 0.0)
                            t_g.append(t3)

                        lhsT = lhsT_by_dl[dl]
                        rows_per_chunk = [8, 8, 8, 8, 3]
                        rc0 = 0
                        for ci, nr in enumerate(rows_per_chunk):
                            rhs = xb[:, rc0 : rc0 + nr, :]
                            for g in range(2):
                                ps = pps.tile([128, 8, Wn], F32, tag="pps")
                                nc.tensor.matmul(
                                    ps[:, :nr, :], lhsT[:, g * 128 : (g + 1) * 128], rhs
                                )
                                a = rs + rc0
                                cp = [nc.scalar.copy, nc.vector.tensor_copy]
                                k = (ci + g) % 2
                                # slot1: standard h2
                                cp[k](
                                    t_g[g][:, 1, a : a + nr, PAD : PAD + Wn],
                                    ps[:, :nr, :],
                                )
                                # slot0: rows shifted +6 (A[r] = s0[r-6])
                                n2 = min(H2ROWS, a + 6 + nr) - (a + 6)
                                if n2 > 0:
                                    cp[1 - k](
                                        t_g[g][:, 0, a + 6 : a + 6 + n2, PAD : PAD + Wn],
                                        ps[:, :n2, :],
                                    )
                                # slot2: cols shifted -6 (s2[r,c] = s0[r,c+6])
                                cp[k](
                                    t_g[g][:, 2, a : a + nr, 0 : Wn - 6 + PAD],
                                    ps[:, :nr, PAD : Wn],
                                )
                            rc0 += nr

                        ring[dl] = t_g

                        if dl >= 6:
                            dl0 = dl - 6  # own-d index 0..15
                            h3t = h3p.tile([HID, HBS * Wn], BF16, tag="h3t")
                            for ci in range(4):  # chunks of 8 output h rows
                                oh = ci * 8
                                gp = []
                                for g in range(2):
                                    ps = dps.tile([128, 512], F32, tag="dps")
                                    first = True
                                    for tzi in range(3):
                                        src = ring[dl + (tzi - 2) * 3][g]
                                        bz = g * 28 + tzi * 9
                                        r_a = oh + PAD + 3
                                        # 3 row-pairs (ty=-1 via slot0, ty=+1
                                        # via slot1) as DoubleRow matmuls
                                        for txi in range(3):
                                            c0 = txi * 3
                                            nc.tensor.matmul(
                                                ps[:],
                                                diag_t[:, bz + 2 * txi : bz + 2 * txi + 2, :],
                                                src[:, 0:2, r_a : r_a + 8, c0 : c0 + Wn],
                                                start=first,
                                                stop=False,
                                                perf_mode=DR,
                                            )
                                            first = False
                                        # col-pair (0,-1)+(0,+1) via slots 1,2
                                        nc.tensor.matmul(
                                            ps[:],
                                            diag_t[:, bz + 6 : bz + 8, :],
                                            src[:, 1:3, oh + PAD : oh + PAD + 8, 0:Wn],
                                            start=False,
                                            stop=False,
                                            perf_mode=DR,
                                        )
                                        # center tap (0,0)
                                        nc.tensor.matmul(
                                            ps[:],
                                            diag_t[:, bz + 8 : bz + 9, :],
                                            src[:, 1:2, oh + PAD : oh + PAD + 8, PAD : PAD + Wn],
                                            start=False,
                                            stop=False,
                                        )
                                    nc.tensor.matmul(
                                        ps[:],
                                        diag_t[:, g * 28 + 27 : g * 28 + 28, :],
                                        ones_t[:],
                                        start=False,
                                        stop=True,
                                    )
                                    gp.append(ps)
                                col = (hb * DCH + dl0) * 4 + ci
                                a_sb = h3p.tile([HID, 512], F32, tag="asb")
                                if ci % 2 == 0:
                                    nc.scalar.copy(a_sb[:], gp[0][:])
                                else:
                                    nc.vector.tensor_copy(a_sb[:], gp[0][:])
                                nc.vector.scalar_tensor_tensor(
                                    h3t[:, oh * Wn : (oh + 8) * Wn],
                                    gp[1][:], 1.0, a_sb[:], mm, mm,
                                    accum_out=pool_cols[:, col : col + 1],
                                )
                            nc.sync.dma_start(
                                h3_dram[:, dl0, hb * HBS * Wn : (hb + 1) * HBS * Wn],
                                h3t[:],
                            )

            # ---- stage 3: SCA pool allreduce -> attn -> fold into post weights ----
            pool_p = sp.tile([HID, 1], F32, tag="poolp")
            nc.vector.tensor_reduce(pool_p[:], pool_cols[:], mybir.AxisListType.X, aa)
            pl_in = dp.tile([HID, 1], F32, tag="plin")
            pl_out = dp.tile([HID, 1], F32, tag="plout")
            nc.sync.dma_start(pl_in[:], pool_p[:])
            if use_collectives:
                nc.gpsimd.collective_compute(
                    "AllReduce", aa, replica_groups=groups,
                    ins=[pl_in.opt()], outs=[pl_out.opt()],
                )
            else:
                nc.sync.dma_start(pl_out[:], pl_in[:])
            pool_f = sp.tile([HID, 1], F32, tag="poolf")
            nc.sync.dma_start(pool_f[:], pl_out[:])
            pool_bf = sp.tile([HID, 1], BF16, tag="poolbf")
            nc.vector.tensor_copy(pool_bf[:], pool_f[:])

            attn = sp.tile([HID, 1], F32, tag="attn")
            with tc.tile_pool(name="scapsum", bufs=1, space="PSUM") as scp:
                aps = scp.tile([HID, 1], F32, tag="aps")
                nc.tensor.matmul(aps[:], scaw_t[:], pool_bf[:])
                nc.scalar.activation(
                    attn[:], aps[:], mybir.ActivationFunctionType.Identity,
                    bias=scab_t[:], scale=1.0,
                )
            post_lhsT = sp.tile([HID, C], BF16, tag="postl")
            nc.vector.tensor_scalar(post_lhsT[:], postw_t[:], attn[:], None, mm)

            # ---- stage 4: post-conv + bias + residual ----
            with (
                tc.tile_pool(name="h3in", bufs=4) as hip,
                tc.tile_pool(name="xres", bufs=4) as xrp,
                tc.tile_pool(name="outp", bufs=4) as op_,
                tc.tile_pool(name="postpsum", bufs=2, space="PSUM") as ppo,
            ):
                for dl0 in range(DCH):
                    h3i = hip.tile([HID, Hn * Wn], BF16, tag="h3i")
                    nc.sync.dma_start(h3i[:], h3_dram[:, dl0, :])
                    xr = xrp.tile([C, Hn, Wn], F32, tag="xr")
                    nc.sync.dma_start(xr[:], x_ext[:, PAD + dl0, :, :])
                    ot = op_.tile([C, Hn * Wn], F32, tag="ot")
                    xrf = xr[:].rearrange("p a b -> p (a b)")
                    for half in range(2):
                        ps = ppo.tile([C, 2048], F32, tag="ppo")
                        for q4 in range(4):
                            c0 = half * 2048 + q4 * 512
                            nc.tensor.matmul(
                                ps[:, q4 * 512 : (q4 + 1) * 512],
                                post_lhsT[:],
                                h3i[:, c0 : c0 + 512],
                            )
                        nc.vector.scalar_tensor_tensor(
                            ot[:, half * 2048 : (half + 1) * 2048],
                            ps[:], pb_t[:], xrf[:, half * 2048 : (half + 1) * 2048],
                            aa, aa,
                        )
                    nc.sync.dma_start(
                        out_ext[:, dl0, :, :],
                        ot[:].rearrange("p (a b) -> p a b", b=Wn),
                    )

    nc.finalize()
    return nc


def _host_prep(inputs):
    x = np.asarray(inputs["x"], np.float32)
    gam = np.asarray(inputs["gn_gamma"], np.float32)
    bet = np.asarray(inputs["gn_beta"], np.float32)
    pre_w = np.asarray(inputs["pre_w"], np.float32)
    pre_b = np.asarray(inputs["pre_b"], np.float32)
    ddc_w = np.asarray(inputs["ddc_w"], np.float32).reshape(H2, KT * KT * KT)
    ddc_b = np.asarray(inputs["ddc_b"], np.float32)
    sca_w = np.asarray(inputs["sca_w"], np.float32)
    sca_b = np.asarray(inputs["sca_b"], np.float32)
    post_w = np.asarray(inputs["post_w"], np.float32)
    post_b = np.asarray(inputs["post_b"], np.float32)

    A = pre_w * gam[None, :]                    # [256, 64]
    at = np.ascontiguousarray(A.T)              # [64, 256]
    qrow = (pre_b + pre_w @ bet)[None, :]       # [1, 256]
    trow = A.sum(axis=1)[None, :]               # [1, 256]

    # diag matrices [128, 56, 128] fp8: per group g, 28 blocks at g*28 + k.
    # Per tz (9 blocks): 3 row-pairs (ty=-1, ty=+1) x tx, col-pair
    # (0,-1),(0,+1), center (0,0); block g*28+27 = bias.
    import ml_dtypes

    diag = np.zeros((128, 56, 128), np.float32)
    idx = np.arange(128)
    for g in range(2):
        w = ddc_w[g * 128 : (g + 1) * 128]  # [128, 27], tap t=(tz+1)*9+(ty+1)*3+(tx+1)
        for tzi in range(3):
            bz = g * 28 + tzi * 9
            for txi in range(3):
                diag[idx, bz + 2 * txi, idx] = w[:, tzi * 9 + 0 * 3 + txi]
                diag[idx, bz + 2 * txi + 1, idx] = w[:, tzi * 9 + 2 * 3 + txi]
            diag[idx, bz + 6, idx] = w[:, tzi * 9 + 3 + 0]
            diag[idx, bz + 7, idx] = w[:, tzi * 9 + 3 + 2]
            diag[idx, bz + 8, idx] = w[:, tzi * 9 + 3 + 1]
        diag[idx, g * 28 + 27, idx] = ddc_b[g * 128 : (g + 1) * 128]

    bf = lambda a: a.astype(ml_dtypes.bfloat16)
    common = {
        "at": at,
        "qrow": qrow,
        "trow": trow,
        "dwdiag": diag.astype(ml_dtypes.float8_e4m3),
        "sca_wT": bf(np.ascontiguousarray((sca_w / NSP).T)),
        "sca_b": sca_b[:, None].astype(np.float32),
        "post_wT": bf(np.ascontiguousarray(post_w.T)),
        "pb": post_b[:, None].astype(np.float32),
    }

    in_maps = []
    for core in range(NCORES):
        b, j = core // JD, core % JD
        lo, hi = j * DCH - PAD, j * DCH + DCH + PAD
        xs = np.zeros((C, DHL, Hn, Wn), np.float32)
        clo, chi = max(lo, 0), min(hi, Dn)
        xs[:, clo - lo : chi - lo] = x[b, :, clo:chi]
        mask = np.ones((1, DHL), np.float32)
        for dl in range(DHL):
            dg = lo + dl
            if dg < 0 or dg >= Dn:
                mask[0, dl] = 0.0
        m = dict(common)
        m["x"] = xs
        m["mask"] = mask
        in_maps.append(m)
    return in_maps


def kernel(**inputs):
    if "nc" not in _CACHED:
        _CACHED["nc"] = _build_nc()
    nc = _CACHED["nc"]
    in_maps = _host_prep(inputs)
    res = run_bass_kernel_spmd(nc, in_maps, list(range(NCORES)))
    out = np.zeros((B, C, Dn, Hn, Wn), np.float32)
    for core in range(NCORES):
        b, j = core // JD, core % JD
        out[b, :, j * DCH : (j + 1) * DCH] = np.asarray(res.results[core]["out"])
    return out

